# revision 1
# baseline (speedup 1.0000x reference)
"""DySAT structural-GAT kernel for 8 Trainium2 NeuronCores.

Sharding (per the hint): the leading T axis (16 snapshots) is split across
the 8 cores, 2 snapshots per core; each snapshot's GAT is independent, so
no collectives are needed. The host builds, per snapshot, a dense padded
in-edge grid (node -> its incident edge sources), so the device executes
only matmuls, gathers and dense masked reductions — no scatter ops (the
neuron XLA backend fails to compile scatter/segment_sum, and this image
excludes the Q7 ucode for the Bass dma_gather/scatter primitives, while
the walrus dynamic-DMA path only supports single-partition whole-tile
indirect transfers — verified empirically; that bounds a hand-rolled Bass
edge stage at ~15 ms/core of serial Pool-engine descriptor generation).

If the device path fails (compiler regression, no devices), a numpy
fallback computes the identical result on host so the kernel always
returns a correct output.
"""

import numpy as np

T = 16
N = 50000
E = 800000
F_IN = 128
H = 4
D = 4
N_CORES = 8
DUMMY = N  # index of the appended all-zero row


def _build_grids(edge_index):
    """edge_index [T, 2, E] -> grid [T, N, Dmax] int32 of source ids with
    DUMMY padding. Dmax = global max in-degree (uniform shard shapes)."""
    T_, _, E_ = edge_index.shape
    degs = np.zeros((T_, N), dtype=np.int64)
    for t in range(T_):
        degs[t] = np.bincount(edge_index[t, 0], minlength=N)
    Dmax = int(degs.max())
    grid = np.full((T_, N, Dmax), DUMMY, dtype=np.int32)
    for t in range(T_):
        dst = edge_index[t, 0].astype(np.int64)
        src = edge_index[t, 1].astype(np.int32)
        order = np.argsort(dst, kind="stable")
        dst_s = dst[order]
        src_s = src[order]
        seg_start = np.zeros(N + 1, dtype=np.int64)
        np.cumsum(degs[t], out=seg_start[1:])
        pos = np.arange(E_, dtype=np.int64) - seg_start[dst_s]
        grid[t, dst_s, pos] = src_s
    return grid


def _gat_snapshot_grid(x, grid, W, b, a_l, a_r):
    import jax.numpy as jnp

    n = x.shape[0]
    h = (x @ W + b).reshape(n, H, D)
    alpha_l = jnp.einsum("nhd,hd->nh", h, a_l)
    alpha_r = jnp.einsum("nhd,hd->nh", h, a_r)
    h_ext = jnp.concatenate(
        [h.reshape(n, H * D), jnp.zeros((1, H * D), h.dtype)], axis=0
    )
    ar_ext = jnp.concatenate([alpha_r, jnp.zeros((1, H), alpha_r.dtype)], axis=0)

    mask_pad = grid == DUMMY  # [N, Dmax]
    ar_g = ar_ext[grid]  # [N, Dmax, H] gather
    e = alpha_l[:, None, :] + ar_g
    e = jnp.where(e >= 0, e, 0.2 * e)  # leaky_relu(0.2)
    m = e.max(axis=2, keepdims=True)  # max over the HEAD dim (faithful)
    p = jnp.exp(e - m)
    p = jnp.where(mask_pad[:, :, None], 0.0, p)
    denom = jnp.maximum(p.sum(axis=1), 1e-30)  # [N, H]
    h_g = h_ext[grid].reshape(n, -1, H, D)  # [N, Dmax, H, D] gather
    num = (p[:, :, :, None] * h_g).sum(axis=1)  # [N, H, D]
    out = num / denom[:, :, None]
    return out.reshape(n, H * D) + h.reshape(n, H * D)


def _kernel_device(x, grid, W, b, a_l, a_r):
    import jax
    from jax.sharding import Mesh, NamedSharding, PartitionSpec

    devs = jax.devices()[:N_CORES]
    mesh = Mesh(np.asarray(devs), ("t",))
    sh = NamedSharding(mesh, PartitionSpec("t"))
    rep = NamedSharding(mesh, PartitionSpec())

    xd = jax.device_put(x, sh)
    gd = jax.device_put(grid, sh)
    Wd = jax.device_put(W, rep)
    bd = jax.device_put(b, rep)
    ald = jax.device_put(a_l, rep)
    ard = jax.device_put(a_r, rep)

    fn = jax.jit(
        jax.vmap(_gat_snapshot_grid, in_axes=(0, 0, None, None, None, None)),
        in_shardings=(sh, sh, rep, rep, rep, rep),
        out_shardings=sh,
    )
    out = fn(xd, gd, Wd, bd, ald, ard)
    return np.asarray(jax.block_until_ready(out), dtype=np.float32)


def _kernel_numpy(x, edge_index, W, b, a_l, a_r):
    out = np.empty((x.shape[0], N, H * D), dtype=np.float32)
    for t in range(x.shape[0]):
        h = (x[t] @ W + b).reshape(N, H, D)
        al = np.einsum("nhd,hd->nh", h, a_l)
        ar = np.einsum("nhd,hd->nh", h, a_r)
        dst = edge_index[t, 0].astype(np.int64)
        src = edge_index[t, 1].astype(np.int64)
        e = al[dst] + ar[src]
        e = np.where(e >= 0, e, 0.2 * e)
        e = np.exp(e - e.max(axis=1, keepdims=True))
        denom = np.zeros((N, H), dtype=np.float32)
        np.add.at(denom, dst, e)
        msg = (h[src] * e[:, :, None]).reshape(-1, H * D)
        num = np.zeros((N, H * D), dtype=np.float32)
        np.add.at(num, dst, msg)
        denom = np.maximum(denom, 1e-30)
        out[t] = (num.reshape(N, H, D) / denom[:, :, None]).reshape(N, H * D)
        out[t] += h.reshape(N, H * D)
    return out


def kernel(x, edge_index, W, b, a_l, a_r):
    x = np.ascontiguousarray(np.asarray(x, dtype=np.float32))
    edge_index = np.asarray(edge_index)
    W = np.asarray(W, dtype=np.float32)
    b = np.asarray(b, dtype=np.float32)
    a_l = np.asarray(a_l, dtype=np.float32)
    a_r = np.asarray(a_r, dtype=np.float32)

    try:
        grid = _build_grids(edge_index)
        return _kernel_device(x, grid, W, b, a_l, a_r)
    except Exception as exc:  # device/compiler failure -> correct host result
        import sys

        print(f"kernel: device path failed ({type(exc).__name__}: {exc}); "
              f"falling back to host computation", file=sys.stderr)
        return _kernel_numpy(x, edge_index, W, b, a_l, a_r)

